# revision 8
# baseline (speedup 1.0000x reference)
"""LongcatFlash MoE kernel for 8 TRN2 NeuronCores (expert-parallel).

Contract: kernel(**inputs) takes the FULL un-sharded inputs from
reference.setup_inputs() and returns the FULL [T, H] output.

Strategy (expert-parallel, memory-regime):
  - Router runs replicated on every core in exact fp32 (top-4 selection
    gaps can be ~3e-7, so the logits matmul stays full-precision). All
    hidden_T chunks are prefetched up-front; expert-weight DMAs are
    issued from the Activation engine so they start only after the
    router's scalar ops, leaving the router the full HBM bandwidth.
  - Experts are sharded 5 weight-slots per core by a host-computed static
    tile schedule (load-balancing metadata only; all routing, indices and
    gatings are computed on device). Hot experts are split across cores
    by token-rank ranges.
  - FFN runs in bf16: weights are converted on host (halves HBM traffic),
    token rows are gathered pre-transposed by the DGE transposing gather
    from a bf16 row copy, both matmuls run at bf16 PE rate, PSUM
    accumulation stays fp32 and the combine scatter-adds bf16 rows.
    All 10 gathers are issued ahead of the compute loop; mm1's PSUM
    double-buffers against the slot-inversion PSUM region so consecutive
    tiles pipeline without a WAR stall.
  - Dispatch bookkeeping (rank prefix-sums + one-hot slot->token
    inversion matmul) is batched across the 10 tiles on the vector
    engine to keep the serial chain short.
  - Zero-experts (ids >= 32) reduce to a per-token scale of the hidden
    row, written by the token-range owner core to a dense per-core
    partial2 block (no scatter, overlaps everything).
  - Host unshards by summing the 8 partial outputs (+ partial2 blocks)
    and undoing the row permutation r = (t % 128) * 16 + t // 128.
"""

import numpy as np

import concourse.bacc as bacc
import concourse.bass as bass
import concourse.mybir as mybir
import concourse.tile as tile
from concourse import library_config
from concourse.bass_utils import run_bass_kernel_spmd

F32 = mybir.dt.float32
BF16 = mybir.dt.bfloat16
I16 = mybir.dt.int16
U32 = mybir.dt.uint32
U8 = mybir.dt.uint8

T, H, I = 2048, 1024, 512
NE, ER, TOPK = 40, 32, 4
ROUTED_SCALE = 2.5
SEL_OFS = 8.0              # masked = 8*selected + score; scores < 8 so the
                           # top-4 of masked are exactly the selected ones
NCORES = 8
NJ = T // 128              # 16 token tiles (r = p*16 + j)
TMAX = 10                  # static FFN tiles per core
NSL = 5                    # weight slots per core
SLOT_CAP = [4, 2, 2, 1, 1]
SLOT_TILES = [[0, 1, 2, 3], [4, 5], [6, 7], [8], [9]]
NSLOT = TMAX * 128         # 1280 dispatch slots per core
AluOp = mybir.AluOpType
ACT_F = mybir.ActivationFunctionType
AXL = mybir.AxisListType


# ---------------------------------------------------------------------------
# host-side schedule
# ---------------------------------------------------------------------------

def _host_routing(hidden, router_w, bias):
    """fp32 routing on host — used ONLY for load-balance scheduling."""
    logits = hidden.astype(np.float32) @ router_w.astype(np.float32).T
    m = logits.max(axis=1, keepdims=True)
    e = np.exp(logits - m)
    scores = e / e.sum(axis=1, keepdims=True)
    biased = scores + bias[None, :]
    ids = np.argsort(-biased, axis=1, kind="stable")[:, :TOPK]
    return ids


def _schedule(ids):
    """Static tile schedule: split-anywhere first-fit-decreasing packing.

    Returns per-core:
      slot_expert[c][s]: global expert id serviced by local weight slot s
      tiles[c][tau]: (expert_id, lo_rank) — dispatch range for FFN tile tau
    Ranks are positions within an expert's selected-token list in r-order.
    """
    counts = np.zeros(ER, np.int64)
    for row in ids:
        for e in row:
            if e < ER:
                counts[e] += 1
    pieces = [[e, 0, (int(counts[e]) + 127) // 128] for e in range(ER)
              if counts[e] > 0]               # [expert, first_tile, ntiles]
    pieces.sort(key=lambda p: -p[2])
    slots = sorted(((SLOT_CAP[s], c, s) for c in range(NCORES)
                    for s in range(NSL)), key=lambda x: -x[0])
    slot_expert = [[0] * NSL for _ in range(NCORES)]
    tiles = [[(0, 1 << 14)] * TMAX for _ in range(NCORES)]
    si = 0
    work = []
    for p in pieces:
        work.append(p)
    while work:
        work.sort(key=lambda p: -p[2])
        p = work.pop(0)
        if si >= len(slots):
            raise RuntimeError("schedule: out of weight slots")
        cap, c, s = slots[si]
        si += 1
        take = min(cap, p[2])
        slot_expert[c][s] = p[0]
        for k in range(cap):
            tau = SLOT_TILES[s][k]
            # tiles beyond `take` extend the range as harmless slack
            tiles[c][tau] = (p[0], 128 * (p[1] + min(k, take)))
        for k in range(take):
            tiles[c][SLOT_TILES[s][k]] = (p[0], 128 * (p[1] + k))
        if p[2] > take:
            work.append([p[0], p[1] + take, p[2] - take])
    return slot_expert, tiles


# ---------------------------------------------------------------------------
# device graph
# ---------------------------------------------------------------------------

_NC_CACHE = {}


def build_nc():
    key = "v4"
    if key in _NC_CACHE:
        return _NC_CACHE[key]
    nc = bacc.Bacc("TRN2", target_bir_lowering=False, debug=False,
                   num_devices=NCORES)

    def din(name, shape, dt):
        return nc.dram_tensor(name, shape, dt, kind="ExternalInput").ap()

    hidden_T = din("hidden_T", [H, T], F32)            # original token order
    hidden_rb = din("hidden_rb", [T, H], BF16)         # r-ordered rows, bf16
    rwt = din("rwt", [H, NE], F32)                     # router_w.T
    bias_b = din("bias_b", [128, NE], F32)             # bias replicated
    w13s = din("w13s", [NSL, H, 2 * I], BF16)          # per-slot [h, i]
    w2s = din("w2s", [NSL, I, H], BF16)                # per-slot [i, h]
    tile_e = din("tile_e", [128, TMAX], F32)           # expert id per tile
    tile_lo = din("tile_lo", [128, TMAX], F32)         # rank range lo per tile
    rhl = din("rhl", [128, NJ, 2], BF16)               # r split (r//128, r%128)
    iota128 = din("iota128", [128, 128], F32)          # row 0..127 replicated
    ident = din("ident", [128, 128], F32)
    identb = din("identb", [128, 128], BF16)
    uts128 = din("uts128", [128, 128], F32)            # strict upper: [k,m]=k<m
    hz = din("hz", [256, H], F32)                      # my zero-path rows

    partial = nc.dram_tensor("partial", [T, H], BF16,
                             kind="ExternalOutput").ap()
    partial2 = nc.dram_tensor("partial2", [256, H], F32,
                              kind="ExternalOutput").ap()

    with tile.TileContext(nc) as tc:
        with (
            tc.tile_pool(name="const", bufs=1) as cpool,
            tc.tile_pool(name="ht", bufs=8) as htpool,
            tc.tile_pool(name="work", bufs=2) as wpool,
            tc.tile_pool(name="persist", bufs=1) as ppool,
            tc.tile_pool(name="wload", bufs=2) as wlpool,
            tc.tile_pool(name="xt", bufs=TMAX) as xtpool,
            tc.tile_pool(name="psum_sm", bufs=1, space="PSUM") as pssm,
            tc.tile_pool(name="psum_big", bufs=2, space="PSUM") as psbig,
            tc.tile_pool(name="psum_y", bufs=1, space="PSUM") as psy,
            tc.tile_pool(name="psum_ib", bufs=1, space="PSUM") as psib,
            tc.tile_pool(name="dram", bufs=1, space="DRAM") as dpool,
        ):
            nc.gpsimd.load_library(library_config.mlp)

            # ---- resident constants ----
            rw_sb = cpool.tile([128, 8, NE], F32, tag="rw")
            nc.sync.dma_start(rw_sb[:], rwt.rearrange("(k p) n -> p k n", p=128))
            bias_sb = cpool.tile([128, 1, NE], F32, tag="bias")
            nc.sync.dma_start(bias_sb[:].rearrange("p a b -> p (a b)"),
                              bias_b[:])
            iota_sb = cpool.tile([128, 1, 128], F32, tag="iota")
            nc.sync.dma_start(iota_sb[:].rearrange("p a b -> p (a b)"),
                              iota128[:])
            ident_sb = cpool.tile([128, 128], F32, tag="ident")
            nc.sync.dma_start(ident_sb[:], ident[:])
            identb_sb = cpool.tile([128, 128], BF16, tag="identb")
            nc.sync.dma_start(identb_sb[:], identb[:])
            uts_sb = cpool.tile([128, 128], F32, tag="uts")
            nc.sync.dma_start(uts_sb[:], uts128[:])
            te_sb = cpool.tile([128, TMAX], F32, tag="te")
            nc.sync.dma_start(te_sb[:], tile_e[:])
            tlo_sb = cpool.tile([128, TMAX], F32, tag="tlo")
            nc.sync.dma_start(tlo_sb[:], tile_lo[:])
            zeros16 = cpool.tile([128, NJ], F32, tag="z16")
            nc.vector.memset(zeros16[:], 0.0)

            # ---- persistent intermediates ----
            idf_all = ppool.tile([128, NJ, TOPK], F32, tag="idf")
            g_all = ppool.tile([128, NJ, TOPK], F32, tag="gall")
            zt_all = ppool.tile([128, NJ, 1], F32, tag="zt")
            lhsT_all = ppool.tile([128, NJ, 2 + TMAX], BF16, tag="lhsT")
            nc.sync.dma_start(lhsT_all[:, :, 0:2], rhl[:])

            # =============== phase 1: router (exact fp32) ===============
            # prefetch ALL hidden_T chunks first; weight DMAs start later
            # (Activation engine) so these get full HBM bandwidth
            ht_tiles = []
            for cq in range(8):
                ht = htpool.tile([128, 8, 256], F32, tag="ht")
                nc.sync.dma_start(
                    ht[:],
                    hidden_T.rearrange("(k p) t -> p k t", p=128)[
                        :, :, cq * 256:(cq + 1) * 256])
                ht_tiles.append(ht)

            for cq in range(8):
                ps_lg = pssm.tile([40, 256], F32, tag="ps_sm", name="ps_lg")
                for k in range(8):
                    nc.tensor.matmul(ps_lg[:], lhsT=rw_sb[:, k, :],
                                     rhs=ht_tiles[cq][:, k, :],
                                     start=(k == 0), stop=(k == 7))
                lgs = wpool.tile([40, 256], F32, tag="lgs")
                nc.vector.tensor_copy(lgs[:], ps_lg[:])
                ps_l = pssm.tile([128, 2, 128], F32, tag="ps_sm", name="ps_lt")
                for q in range(2):
                    nc.tensor.transpose(ps_l[:, q, :NE],
                                        lgs[:, q * 128:(q + 1) * 128],
                                        ident_sb[:NE, :NE])
                ps_l = ps_l[:, :, :NE]
                # batched softmax over both q tiles
                rmax = wpool.tile([128, 2, 1], F32, tag="rmax")
                nc.vector.tensor_reduce(rmax[:], ps_l, axis=AXL.X,
                                        op=AluOp.max, negate=True)  # -max
                ex = wpool.tile([128, 2, NE], F32, tag="ex")
                for q in range(2):
                    nc.scalar.activation(ex[:, q, :], ps_l[:, q, :],
                                         ACT_F.Exp, bias=rmax[:, q, 0:1])
                rsum = wpool.tile([128, 2, 1], F32, tag="rsum")
                nc.vector.tensor_reduce(rsum[:], ex[:], axis=AXL.X,
                                        op=AluOp.add)
                rinv = wpool.tile([128, 2, 1], F32, tag="rinv")
                nc.vector.reciprocal(rinv[:], rsum[:])
                scores = wpool.tile([128, 2, NE], F32, tag="scores")
                nc.vector.tensor_tensor(scores[:], ex[:],
                                        rinv[:].to_broadcast([128, 2, NE]),
                                        op=AluOp.mult)
                biased = wpool.tile([128, 2, NE], F32, tag="biased")
                nc.vector.tensor_tensor(biased[:], scores[:],
                                        bias_sb[:].to_broadcast([128, 2, NE]),
                                        op=AluOp.add)
                # per-q top-4 selection
                for q in range(2):
                    j = 2 * cq + q
                    top8 = wpool.tile([128, 8], F32, tag="top8")
                    nc.vector.max(top8[:], biased[:, q, :])
                    pred = wpool.tile([128, NE], F32, tag="pred")
                    nc.vector.tensor_scalar(pred[:], biased[:, q, :],
                                            top8[:, 3:4], None,
                                            op0=AluOp.is_ge)
                    masked = wpool.tile([128, NE], F32, tag="masked")
                    nc.vector.scalar_tensor_tensor(
                        masked[:], pred[:], SEL_OFS, scores[:, q, :],
                        op0=AluOp.mult, op1=AluOp.add)
                    vals8 = wpool.tile([128, 8], F32, tag="vals8")
                    nc.vector.max(vals8[:], masked[:])
                    idx8 = wpool.tile([128, 8], U32, tag="idx8")
                    nc.vector.max_index(idx8[:], vals8[:], masked[:])
                    nc.vector.tensor_scalar(g_all[:, j, :], vals8[:, :TOPK],
                                            -SEL_OFS, ROUTED_SCALE,
                                            op0=AluOp.add, op1=AluOp.mult)
                    nc.vector.tensor_copy(idf_all[:, j, :], idx8[:, :TOPK])
                # batched zero-expert gating total over both q
                zm = wpool.tile([128, 2, TOPK], F32, tag="zm")
                nc.vector.tensor_scalar(zm[:], idf_all[:, 2 * cq:2 * cq + 2, :],
                                        ER - 0.5, None, op0=AluOp.is_gt)
                nc.vector.tensor_mul(zm[:], zm[:],
                                     g_all[:, 2 * cq:2 * cq + 2, :])
                nc.vector.tensor_reduce(zt_all[:, 2 * cq:2 * cq + 2, :],
                                        zm[:], axis=AXL.X, op=AluOp.add)

            zt_flat = dpool.tile([1, T], F32, tag="ztflat")
            nc.sync.dma_start(zt_flat[0, :].rearrange("(p j) -> p j", p=128),
                              zt_all[:].rearrange("p j o -> p (j o)"))

            # ====== phase 1.5: zero-expert path (dense write, no DGE) ======
            pid = nc.sync.partition_id()
            for tt in range(2):
                ztv = wpool.tile([1, 128], F32, tag="ztv")
                nc.sync.dma_start(
                    ztv[:], zt_flat[0:1, bass.ds(pid * 256 + tt * 128, 128)])
                ps_zt = pssm.tile([128, 1], F32, tag="ps_sm", name="ps_zt")
                nc.tensor.transpose(ps_zt[:], ztv[:], ident_sb[:1, :1])
                ztc = wpool.tile([128, 1], F32, tag="ztc")
                nc.vector.tensor_copy(ztc[:], ps_zt[:])
                hzt = wpool.tile([128, H], F32, tag="hzt")
                nc.sync.dma_start(hzt[:], hz[tt * 128:(tt + 1) * 128, :])
                yz = wpool.tile([128, H], F32, tag="yz")
                nc.scalar.activation(yz[:], hzt[:], ACT_F.Copy,
                                     scale=ztc[:, 0:1])
                nc.sync.dma_start(partial2[tt * 128:(tt + 1) * 128, :], yz[:])

            # ========= phase 2: dispatch bookkeeping (tile-batched) =========
            mask_all = ppool.tile([128, TMAX, NJ], F32, tag="maskall")
            inrow_all = ppool.tile([128, TMAX, NJ], F32, tag="inrowall")
            gvt = ppool.tile([128, TMAX, NJ], F32, tag="gvt")
            for j in range(NJ):
                eq = wpool.tile([128, TMAX, TOPK], F32, tag="eq")
                nc.vector.tensor_tensor(
                    eq[:], idf_all[:, j:j + 1, :].to_broadcast(
                        [128, TMAX, TOPK]),
                    te_sb[:].to_broadcast([128, TMAX, TOPK]),
                    op=AluOp.is_equal)
                gv = wpool.tile([128, TMAX, TOPK], F32, tag="gvx")
                nc.vector.tensor_mul(
                    gv[:], eq[:],
                    g_all[:, j:j + 1, :].to_broadcast([128, TMAX, TOPK]))
                nc.vector.tensor_reduce(mask_all[:, :, j:j + 1], eq[:],
                                        axis=AXL.X, op=AluOp.max)
                nc.vector.tensor_reduce(gvt[:, :, j:j + 1], gv[:],
                                        axis=AXL.X, op=AluOp.add)
            # gatings into the one-hot lhsT (transposed view write)
            for tau in range(TMAX):
                nc.vector.tensor_copy(lhsT_all[:, :, 2 + tau],
                                      gvt[:, tau, :])
            # ranks: rowsum per (tau), prefix over partitions, scan over j
            rowsum_all = wpool.tile([128, TMAX, 1], F32, tag="rowsumall")
            nc.vector.tensor_reduce(rowsum_all[:], mask_all[:],
                                    axis=AXL.X, op=AluOp.add)
            ps_rp = pssm.tile([128, TMAX], F32, tag="ps_sm", name="ps_rp")
            nc.tensor.matmul(ps_rp[:], lhsT=uts_sb[:],
                             rhs=rowsum_all[:].rearrange("p t o -> p (t o)"),
                             start=True, stop=True)
            rp_sb = wpool.tile([128, TMAX, 1], F32, tag="rpsb")
            nc.vector.tensor_copy(rp_sb[:].rearrange("p t o -> p (t o)"),
                                  ps_rp[:])
            for tau in range(TMAX):
                nc.vector.tensor_tensor_scan(
                    inrow_all[:, tau, :], mask_all[:, tau, :], zeros16[:],
                    0.0, op0=AluOp.add, op1=AluOp.add)
            pos = wpool.tile([128, TMAX, NJ], F32, tag="pos")
            nc.vector.tensor_tensor(
                pos[:], inrow_all[:],
                rp_sb[:].to_broadcast([128, TMAX, NJ]), op=AluOp.add)
            nc.vector.tensor_sub(pos[:], pos[:], mask_all[:])
            t1 = wpool.tile([128, TMAX, NJ], F32, tag="t1")
            nc.vector.tensor_tensor(
                t1[:], pos[:], tlo_sb[:].to_broadcast([128, TMAX, NJ]),
                op=AluOp.subtract)
            okr = wpool.tile([128, TMAX, NJ], F32, tag="okr")
            nc.vector.tensor_scalar(okr[:], t1[:], -0.5, None, op0=AluOp.is_gt)
            ok2 = wpool.tile([128, TMAX, NJ], F32, tag="ok2")
            nc.vector.tensor_scalar(ok2[:], t1[:], 127.5, None, op0=AluOp.is_lt)
            nc.vector.tensor_mul(okr[:], okr[:], ok2[:])
            nc.vector.tensor_mul(okr[:], okr[:], mask_all[:])
            oku = wpool.tile([128, TMAX, NJ], U8, tag="oku")
            nc.vector.tensor_copy(oku[:], okr[:])
            slotm = ppool.tile([128, TMAX, NJ], F32, tag="slotm")
            nc.vector.memset(slotm[:], -4.0)
            nc.vector.copy_predicated(slotm[:], oku[:], t1[:])

            # one-hot build + inversion matmul (PSUM: 2+1 banks, the big
            # 2-bank half shares the double-buffered pool with mm1's gu)
            invA = psbig.tile([2 + TMAX, 1024], F32, tag="ps_big",
                              name="ps_invA")
            invB = psib.tile([2 + TMAX, NSLOT - 1024], F32, tag="ps_ib")
            for j in range(NJ):
                oh = wpool.tile([128, TMAX, 128], BF16, tag="oh")
                nc.vector.tensor_tensor(
                    oh[:], iota_sb[:].to_broadcast([128, TMAX, 128]),
                    slotm[:, :, j:j + 1].to_broadcast([128, TMAX, 128]),
                    op=AluOp.is_equal)
                ohf = oh[:].rearrange("p a b -> p (a b)")
                for ci, (lo, hi) in enumerate([(0, 512), (512, 1024),
                                               (1024, NSLOT)]):
                    dst = invA[:, lo:hi] if hi <= 1024 else \
                        invB[:, 0:NSLOT - 1024]
                    nc.tensor.matmul(dst, lhsT=lhsT_all[:, j, :],
                                     rhs=ohf[:, lo:hi],
                                     start=(j == 0), stop=(j == NJ - 1))

            # decode via PE transpose: tsp[p, tau, :] = inv[:, tau*128+p]
            inv_sb = wpool.tile([2 + TMAX, NSLOT], F32, tag="invsb")
            nc.vector.tensor_copy(inv_sb[:, 0:1024], invA[:])
            nc.vector.tensor_copy(inv_sb[:, 1024:NSLOT], invB[:])
            tsp_sb = ppool.tile([128, TMAX, 2 + TMAX], F32, tag="tsp")
            for tau in range(TMAX):
                ps_tsp = pssm.tile([128, 128], F32, tag="ps_sm", name="ps_tsp")
                nc.tensor.transpose(ps_tsp[:, :2 + TMAX],
                                    inv_sb[:, tau * 128:(tau + 1) * 128],
                                    ident_sb[:2 + TMAX, :2 + TMAX])
                nc.vector.tensor_copy(tsp_sb[:, tau, :], ps_tsp[:, :2 + TMAX])
            # r = 128*hi + lo  (per-partition layout [p, tau])
            r_pt = ppool.tile([128, TMAX], F32, tag="rpt")
            nc.vector.scalar_tensor_tensor(r_pt[:], tsp_sb[:, :, 0], 128.0,
                                           tsp_sb[:, :, 1], op0=AluOp.mult,
                                           op1=AluOp.add)
            g_wr = ppool.tile([128, TMAX], F32, tag="gwr")
            for tau in range(TMAX):
                nc.vector.tensor_copy(g_wr[:, tau:tau + 1],
                                      tsp_sb[:, tau, 2 + tau:3 + tau])
            # int16 slot->token list to DRAM, reload 16-wrapped for the DGE ops
            r_i16 = wpool.tile([128, TMAX], I16, tag="ri16")
            nc.vector.tensor_copy(r_i16[:], r_pt[:])
            slots_dram = dpool.tile([1, NSLOT], I16, tag="slotsdram")
            nc.sync.dma_start(
                slots_dram[0, :].rearrange("(t p) -> p t", p=128), r_i16[:])
            idxw = ppool.tile([128, NSLOT // 16], I16, tag="idxw")
            for grp in range(8):
                nc.sync.dma_start(
                    idxw[grp * 16:(grp + 1) * 16, :],
                    slots_dram[0, :].rearrange("(c p) -> p c", p=16))

            # =============== phase 3: gather + FFN + combine ===============
            # all transposing gathers first (gpsimd program order => no
            # per-tile gather stall)
            xt_tiles = []
            for tau in range(TMAX):
                xt = xtpool.tile([128, 8, 128], BF16, tag="xt")
                nc.gpsimd.dma_gather(
                    out_ap=xt[:], in_ap=hidden_rb[:],
                    idxs_ap=idxw[:, tau * 8:(tau + 1) * 8],
                    num_idxs=128, num_idxs_reg=128, elem_size=H,
                    transpose=True)
                xt_tiles.append(xt)

            for s in range(NSL):
                w13_sb = wlpool.tile([128, 8, 2 * I], BF16, tag="w13")
                w2_sb = wlpool.tile([128, 4, H], BF16, tag="w2")
                # issued on the Activation engine: starts after the router's
                # scalar ops, i.e. does not compete with the hidden_T loads
                nc.scalar.dma_start(w13_sb[:],
                                    w13s[s].rearrange("(k p) i -> p k i",
                                                      p=128))
                nc.scalar.dma_start(w2_sb[:],
                                    w2s[s].rearrange("(k p) i -> p k i",
                                                     p=128))

                for tau in SLOT_TILES[s]:
                    xt = xt_tiles[tau]
                    # mm1
                    ps_gu = psbig.tile([128, 2 * I], F32, tag="ps_big",
                                       name="ps_gu")
                    for k in range(8):
                        for n in range(2):
                            nc.tensor.matmul(ps_gu[:, n * 512:(n + 1) * 512],
                                             lhsT=xt[:, k, :],
                                             rhs=w13_sb[:, k, n * 512:(n + 1) * 512],
                                             start=(k == 0), stop=(k == 7))
                    # h = silu(gate) * up = gate * sigmoid(gate) * up
                    sg = wpool.tile([128, I], F32, tag="sg")
                    nc.scalar.activation(sg[:], ps_gu[:, :I], ACT_F.Sigmoid)
                    nc.vector.tensor_mul(sg[:], sg[:], ps_gu[:, :I])
                    hh = wpool.tile([128, I], BF16, tag="hh")
                    nc.vector.tensor_mul(hh[:], sg[:], ps_gu[:, I:])
                    ht2 = wpool.tile([128, 4, 128], BF16, tag="ht2")
                    for k in range(4):
                        ps_t2 = pssm.tile([128, 128], BF16, tag="ps_sm",
                                          name="ps_t2")
                        nc.tensor.transpose(ps_t2[:],
                                            hh[:, k * 128:(k + 1) * 128],
                                            identb_sb[:])
                        nc.vector.tensor_copy(ht2[:, k, :], ps_t2[:])
                    # mm2
                    ps_y = psy.tile([128, H], F32, tag="ps_y")
                    for k in range(4):
                        for n in range(2):
                            nc.tensor.matmul(ps_y[:, n * 512:(n + 1) * 512],
                                             lhsT=ht2[:, k, :],
                                             rhs=w2_sb[:, k, n * 512:(n + 1) * 512],
                                             start=(k == 0), stop=(k == 3))
                    yv = wpool.tile([128, 1, H], BF16, tag="yv")
                    nc.scalar.activation(yv[:, 0, :], ps_y[:], ACT_F.Copy,
                                         scale=g_wr[:, tau:tau + 1])
                    nc.gpsimd.dma_scatter_add(
                        out_ap=partial[:], in_ap=yv[:],
                        idxs_ap=idxw[:, tau * 8:(tau + 1) * 8],
                        num_idxs=128, num_idxs_reg=128, elem_size=H)

    nc.compile()
    _NC_CACHE[key] = nc
    return nc


# ---------------------------------------------------------------------------
# host wrapper
# ---------------------------------------------------------------------------

def make_in_maps(hidden_states, router_w, e_score_correction_bias, w13, w2):
    import ml_dtypes
    hidden_states = np.asarray(hidden_states, np.float32)
    router_w = np.asarray(router_w, np.float32)
    bias = np.asarray(e_score_correction_bias, np.float32)
    w13 = np.asarray(w13, np.float32)
    w2 = np.asarray(w2, np.float32)

    ids = _host_routing(hidden_states, router_w, bias)
    slot_expert, tiles = _schedule(ids)

    # r = (t % 128) * 16 + t // 128  <->  t = (r % 16) * 128 + r // 16
    r_of_t = (np.arange(T) % 128) * 16 + np.arange(T) // 128
    t_of_r = np.empty(T, np.int64)
    t_of_r[r_of_t] = np.arange(T)

    hidden_T = np.ascontiguousarray(hidden_states.T)
    hidden_rows = np.ascontiguousarray(hidden_states[t_of_r])
    hidden_rb = hidden_rows.astype(ml_dtypes.bfloat16)
    rwt = np.ascontiguousarray(router_w.T)
    bias_b = np.tile(bias[None, :], (128, 1))
    w13t = np.ascontiguousarray(
        w13.transpose(0, 2, 1)).astype(ml_dtypes.bfloat16)   # [e, h, 2I]
    w2t = np.ascontiguousarray(
        w2.transpose(0, 2, 1)).astype(ml_dtypes.bfloat16)    # [e, i, h]

    rr = np.arange(T).reshape(128, NJ).astype(np.float32)  # r at [p, j]
    rhl = np.stack([rr // 128, rr % 128], axis=-1).astype(ml_dtypes.bfloat16)
    iota128 = np.tile(np.arange(128, dtype=np.float32), (128, 1))
    ident = np.eye(128, dtype=np.float32)
    identb = np.eye(128, dtype=ml_dtypes.bfloat16)
    uts128 = np.triu(np.ones((128, 128), np.float32), k=1)

    in_maps = []
    for c in range(NCORES):
        te = np.array([tiles[c][tau][0] for tau in range(TMAX)], np.float32)
        tlo = np.array([tiles[c][tau][1] for tau in range(TMAX)], np.float32)
        in_maps.append({
            "hidden_T": hidden_T,
            "hidden_rb": hidden_rb,
            "rwt": rwt,
            "bias_b": bias_b,
            "w13s": np.ascontiguousarray(
                w13t[[slot_expert[c][s] for s in range(NSL)]]),
            "w2s": np.ascontiguousarray(
                w2t[[slot_expert[c][s] for s in range(NSL)]]),
            "tile_e": np.tile(te[None, :], (128, 1)),
            "tile_lo": np.tile(tlo[None, :], (128, 1)),
            "rhl": rhl,
            "iota128": iota128,
            "ident": ident,
            "identb": identb,
            "uts128": uts128,
            "hz": np.ascontiguousarray(hidden_rows[c * 256:(c + 1) * 256]),
        })
    return in_maps, t_of_r


def kernel(hidden_states, router_w, e_score_correction_bias, w13, w2,
           _trace=False):
    nc = build_nc()
    in_maps, t_of_r = make_in_maps(hidden_states, router_w,
                                   e_score_correction_bias, w13, w2)
    res = run_bass_kernel_spmd(nc, in_maps, core_ids=list(range(NCORES)),
                               trace=_trace)
    total = np.zeros((T, H), np.float64)
    for c in range(NCORES):
        total += res.results[c]["partial"].astype(np.float64)
        total[c * 256:(c + 1) * 256] += \
            res.results[c]["partial2"].astype(np.float64)
    out = np.empty((T, H), np.float32)
    out[t_of_r] = total.astype(np.float32)      # out[t] = total[r(t)]
    kernel._last_results = res
    return out


# revision 14
# speedup vs baseline: 1.1621x; 1.1621x over previous
"""LongcatFlash MoE kernel for 8 TRN2 NeuronCores (expert-parallel).

Contract: kernel(**inputs) takes the FULL un-sharded inputs from
reference.setup_inputs() and returns the FULL [T, H] output.

Strategy (expert-parallel, memory-regime):
  - Router runs replicated on every core in exact fp32 (top-4 selection
    gaps can be ~3e-7, so the logits matmul stays full-precision). The
    top-4 selection splits work across the vector and gpsimd engines.
  - Experts are sharded into per-core weight slots by a host-computed
    input-dependent tile schedule (load-balancing metadata only; all
    routing, indices and gatings are computed on device). Hot experts
    are split across cores by token-rank ranges. The tile count adapts
    to the input (9 tiles/core for balanced loads, 10 as fallback).
  - FFN runs in bf16: weights are converted on host (halves HBM traffic),
    token rows are gathered pre-transposed by the DGE transposing gather
    from a bf16 row copy, both matmuls run at bf16 PE rate, PSUM
    accumulation stays fp32 and the combine scatter-adds bf16 rows.
    All gathers are issued ahead of the compute loop; mm1's PSUM
    double-buffers against the slot-inversion PSUM region so consecutive
    tiles pipeline without a WAR stall.
  - Dispatch bookkeeping (rank prefix-sums + one-hot slot->token
    inversion matmul) is batched across tiles on the vector engine; the
    slot list is written to DRAM directly in the 16-wrapped DGE index
    layout so the reload is plain replication.
  - Zero-experts (ids >= 32) reduce to a per-token scale of the hidden
    row, written by the token-range owner core to a dense per-core
    partial2 block (no scatter, overlaps everything).
  - Host unshards by summing the 8 partial outputs (+ partial2 blocks)
    and undoing the row permutation r = (t % 128) * 16 + t // 128.
"""

import numpy as np

import concourse.bacc as bacc
import concourse.bass as bass
import concourse.mybir as mybir
import concourse.tile as tile
from concourse import library_config
from concourse.bass_utils import run_bass_kernel_spmd

F32 = mybir.dt.float32
BF16 = mybir.dt.bfloat16
I16 = mybir.dt.int16
U32 = mybir.dt.uint32
U8 = mybir.dt.uint8

T, H, I = 2048, 1024, 512
NE, ER, TOPK = 40, 32, 4
ROUTED_SCALE = 2.5
SEL_OFS = 8.0              # masked = 8*selected + score; scores < 8 so the
                           # top-4 of masked are exactly the selected ones
NCORES = 8
NJ = T // 128              # 16 token tiles (r = p*16 + j)
CAPS9 = [4, 2, 1, 1, 1]
CAPS10 = [4, 2, 2, 1, 1]
AluOp = mybir.AluOpType
ACT_F = mybir.ActivationFunctionType
AXL = mybir.AxisListType


# ---------------------------------------------------------------------------
# host-side schedule
# ---------------------------------------------------------------------------

def _host_routing(hidden, router_w, bias):
    """fp32 routing on host — used ONLY for load-balance scheduling."""
    logits = hidden.astype(np.float32) @ router_w.astype(np.float32).T
    m = logits.max(axis=1, keepdims=True)
    e = np.exp(logits - m)
    scores = e / e.sum(axis=1, keepdims=True)
    biased = scores + bias[None, :]
    ids = np.argsort(-biased, axis=1, kind="stable")[:, :TOPK]
    return ids


def _schedule(ids):
    """Input-adaptive tile schedule: split-anywhere first-fit-decreasing.

    Returns (caps, slot_expert, tiles):
      caps: per-slot tile capacities (same structure on every core)
      slot_expert[c][s]: global expert id serviced by local weight slot s
      tiles[c][tau]: (expert_id, lo_rank) — dispatch range for FFN tile tau
    Ranks are positions within an expert's selected-token list in r-order.
    """
    counts = np.zeros(ER, np.int64)
    for row in ids:
        for e in row:
            if e < ER:
                counts[e] += 1
    pieces0 = [[e, 0, (int(counts[e]) + 127) // 128] for e in range(ER)
               if counts[e] > 0]              # [expert, first_tile, ntiles]

    for caps in (CAPS9, CAPS10):
        tmax = sum(caps)
        slot_tiles = []
        t0 = 0
        for cap in caps:
            slot_tiles.append(list(range(t0, t0 + cap)))
            t0 += cap
        slots = sorted(((cap, c, s) for c in range(NCORES)
                        for s, cap in enumerate(caps)), key=lambda x: -x[0])
        slot_expert = [[0] * len(caps) for _ in range(NCORES)]
        tiles = [[(0, 1 << 14)] * tmax for _ in range(NCORES)]
        si = 0
        work = [list(p) for p in pieces0]
        ok = True
        while work:
            work.sort(key=lambda p: -p[2])
            p = work.pop(0)
            if si >= len(slots):
                ok = False
                break
            cap, c, s = slots[si]
            si += 1
            take = min(cap, p[2])
            slot_expert[c][s] = p[0]
            for k in range(cap):
                tau = slot_tiles[s][k]
                # tiles beyond `take` extend the range as harmless slack
                tiles[c][tau] = (p[0], 128 * (p[1] + min(k, take)))
            for k in range(take):
                tiles[c][slot_tiles[s][k]] = (p[0], 128 * (p[1] + k))
            if p[2] > take:
                work.append([p[0], p[1] + take, p[2] - take])
        if ok:
            return caps, slot_expert, tiles
    raise RuntimeError("schedule: out of weight slots")


# ---------------------------------------------------------------------------
# device graph
# ---------------------------------------------------------------------------

_NC_CACHE = {}


def build_nc(caps):
    key = tuple(caps)
    if key in _NC_CACHE:
        return _NC_CACHE[key]

    TMAX = sum(caps)
    NSL = len(caps)
    SLOT_TILES = []
    t0 = 0
    for cap in caps:
        SLOT_TILES.append(list(range(t0, t0 + cap)))
        t0 += cap
    NSLOT = TMAX * 128
    NB = NSLOT - 1024           # tail chunk of the inversion PSUM

    nc = bacc.Bacc("TRN2", target_bir_lowering=False, debug=False,
                   num_devices=NCORES)

    def din(name, shape, dt):
        return nc.dram_tensor(name, shape, dt, kind="ExternalInput").ap()

    hidden_T = din("hidden_T", [H, T], F32)            # original token order
    hidden_rb = din("hidden_rb", [T, H], BF16)         # r-ordered rows, bf16
    rwt = din("rwt", [H, NE], F32)                     # router_w.T
    bias_b = din("bias_b", [128, NE], F32)             # bias replicated
    w13s = din("w13s", [NSL, H, 2 * I], BF16)          # per-slot [h, i]
    w2s = din("w2s", [NSL, I, H], BF16)                # per-slot [i, h]
    tile_e = din("tile_e", [128, TMAX], F32)           # expert id per tile
    tile_lo = din("tile_lo", [128, TMAX], F32)         # rank range lo per tile
    rhl = din("rhl", [128, NJ, 2], BF16)               # r split (r//128, r%128)
    iota128 = din("iota128", [128, 128], F32)          # row 0..127 replicated
    ident = din("ident", [128, 128], F32)
    identb = din("identb", [128, 128], BF16)
    uts128 = din("uts128", [128, 128], F32)            # strict upper: [k,m]=k<m
    hz = din("hz", [256, H], F32)                      # my zero-path rows

    partial = nc.dram_tensor("partial", [T, H], BF16,
                             kind="ExternalOutput").ap()
    partial2 = nc.dram_tensor("partial2", [256, H], F32,
                              kind="ExternalOutput").ap()

    with tile.TileContext(nc) as tc:
        with (
            tc.tile_pool(name="const", bufs=1) as cpool,
            tc.tile_pool(name="ht", bufs=6) as htpool,
            tc.tile_pool(name="work", bufs=2) as wpool,
            tc.tile_pool(name="persist", bufs=1) as ppool,
            tc.tile_pool(name="w13l", bufs=3) as w13pool,
            tc.tile_pool(name="w2l", bufs=3) as w2pool,
            tc.tile_pool(name="xt", bufs=TMAX) as xtpool,
            tc.tile_pool(name="psum_sm", bufs=1, space="PSUM") as pssm,
            tc.tile_pool(name="psum_big", bufs=2, space="PSUM") as psbig,
            tc.tile_pool(name="psum_y", bufs=1, space="PSUM") as psy,
            tc.tile_pool(name="psum_ib", bufs=1, space="PSUM") as psib,
            tc.tile_pool(name="dram", bufs=1, space="DRAM") as dpool,
        ):
            nc.gpsimd.load_library(library_config.mlp)

            # ---- resident constants ----
            rw_sb = cpool.tile([128, 8, NE], F32, tag="rw")
            nc.sync.dma_start(rw_sb[:], rwt.rearrange("(k p) n -> p k n", p=128))
            bias_sb = cpool.tile([128, 1, NE], F32, tag="bias")
            nc.sync.dma_start(bias_sb[:].rearrange("p a b -> p (a b)"),
                              bias_b[:])
            iota_sb = cpool.tile([128, 1, 128], F32, tag="iota")
            nc.sync.dma_start(iota_sb[:].rearrange("p a b -> p (a b)"),
                              iota128[:])
            ident_sb = cpool.tile([128, 128], F32, tag="ident")
            nc.sync.dma_start(ident_sb[:], ident[:])
            identb_sb = cpool.tile([128, 128], BF16, tag="identb")
            nc.sync.dma_start(identb_sb[:], identb[:])
            uts_sb = cpool.tile([128, 128], F32, tag="uts")
            nc.sync.dma_start(uts_sb[:], uts128[:])
            te_sb = cpool.tile([128, TMAX, 1], F32, tag="te")
            nc.sync.dma_start(te_sb[:].rearrange("p t o -> p (t o)"),
                              tile_e[:])
            tlo_sb = cpool.tile([128, TMAX], F32, tag="tlo")
            nc.sync.dma_start(tlo_sb[:], tile_lo[:])
            zeros16 = cpool.tile([128, NJ], F32, tag="z16")
            nc.vector.memset(zeros16[:], 0.0)

            # ---- persistent intermediates ----
            idf_all = ppool.tile([128, 1, NJ, TOPK], F32, tag="idf")
            g_all = ppool.tile([128, 1, NJ, TOPK], F32, tag="gall")
            zt_all = ppool.tile([128, NJ, 1], F32, tag="zt")
            lhsT_all = ppool.tile([128, NJ, 2 + TMAX], BF16, tag="lhsT")
            nc.sync.dma_start(lhsT_all[:, :, 0:2], rhl[:])

            # =============== phase 1: router (exact fp32) ===============
            ht_tiles = []
            for cq in range(8):
                ht = htpool.tile([128, 8, 256], F32, tag="ht")
                nc.sync.dma_start(
                    ht[:],
                    hidden_T.rearrange("(k p) t -> p k t", p=128)[
                        :, :, cq * 256:(cq + 1) * 256])
                ht_tiles.append(ht)

            for cq in range(8):
                ps_lg = pssm.tile([40, 256], F32, tag="ps_sm", name="ps_lg")
                for k in range(8):
                    nc.tensor.matmul(ps_lg[:], lhsT=rw_sb[:, k, :],
                                     rhs=ht_tiles[cq][:, k, :],
                                     start=(k == 0), stop=(k == 7))
                lgs = wpool.tile([40, 256], F32, tag="lgs")
                nc.vector.tensor_copy(lgs[:], ps_lg[:])
                ps_l = pssm.tile([128, 2, 128], F32, tag="ps_sm", name="ps_lt")
                for q in range(2):
                    nc.tensor.transpose(ps_l[:, q, :NE],
                                        lgs[:, q * 128:(q + 1) * 128],
                                        ident_sb[:NE, :NE])
                ps_l = ps_l[:, :, :NE]
                # batched softmax over both q tiles
                rmax = wpool.tile([128, 2, 1], F32, tag="rmax")
                nc.vector.tensor_reduce(rmax[:], ps_l, axis=AXL.X,
                                        op=AluOp.max, negate=True)  # -max
                ex = wpool.tile([128, 2, NE], F32, tag="ex")
                for q in range(2):
                    nc.scalar.activation(ex[:, q, :], ps_l[:, q, :],
                                         ACT_F.Exp, bias=rmax[:, q, 0:1])
                rsum = wpool.tile([128, 2, 1], F32, tag="rsum")
                nc.vector.tensor_reduce(rsum[:], ex[:], axis=AXL.X,
                                        op=AluOp.add)
                rinv = wpool.tile([128, 2, 1], F32, tag="rinv")
                nc.vector.reciprocal(rinv[:], rsum[:])
                scores = wpool.tile([128, 2, NE], F32, tag="scores")
                nc.vector.tensor_tensor(scores[:], ex[:],
                                        rinv[:].to_broadcast([128, 2, NE]),
                                        op=AluOp.mult)
                biased = wpool.tile([128, 2, NE], F32, tag="biased")
                nc.vector.tensor_tensor(biased[:], scores[:],
                                        bias_sb[:].to_broadcast([128, 2, NE]),
                                        op=AluOp.add)
                # per-q top-4 selection (DVE max ops, gpsimd side work)
                for q in range(2):
                    j = 2 * cq + q
                    top8 = wpool.tile([128, 8], F32, tag="top8")
                    nc.vector.max(top8[:], biased[:, q, :])
                    pred = wpool.tile([128, NE], F32, tag="pred")
                    nc.vector.tensor_scalar(pred[:], biased[:, q, :],
                                            top8[:, 3:4], None,
                                            op0=AluOp.is_ge)
                    masked = wpool.tile([128, NE], F32, tag="masked")
                    nc.vector.scalar_tensor_tensor(
                        masked[:], pred[:], SEL_OFS, scores[:, q, :],
                        op0=AluOp.mult, op1=AluOp.add)
                    vals8 = wpool.tile([128, 8], F32, tag="vals8")
                    nc.vector.max(vals8[:], masked[:])
                    idx8 = wpool.tile([128, 8], U32, tag="idx8")
                    nc.vector.max_index(idx8[:], vals8[:], masked[:])
                    nc.vector.tensor_scalar(g_all[:, 0, j, :], vals8[:, :TOPK],
                                            -SEL_OFS, ROUTED_SCALE,
                                            op0=AluOp.add, op1=AluOp.mult)
                    nc.gpsimd.tensor_copy(idf_all[:, 0, j, :], idx8[:, :TOPK])
                # batched zero-expert gating total over both q
                zm = wpool.tile([128, 2, TOPK], F32, tag="zm")
                nc.vector.tensor_scalar(zm[:],
                                        idf_all[:, 0, 2 * cq:2 * cq + 2, :],
                                        ER - 0.5, None, op0=AluOp.is_gt)
                nc.gpsimd.tensor_mul(zm[:], zm[:],
                                     g_all[:, 0, 2 * cq:2 * cq + 2, :])
                nc.vector.tensor_reduce(zt_all[:, 2 * cq:2 * cq + 2, :],
                                        zm[:], axis=AXL.X, op=AluOp.add)

            zt_flat = dpool.tile([1, T], F32, tag="ztflat")
            nc.sync.dma_start(zt_flat[0, :].rearrange("(p j) -> p j", p=128),
                              zt_all[:].rearrange("p j o -> p (j o)"))

            # ====== phase 1.5: zero-expert path (dense write, no DGE) ======
            pid = nc.sync.partition_id()
            for tt in range(2):
                ztv = wpool.tile([1, 128], F32, tag="ztv")
                nc.sync.dma_start(
                    ztv[:], zt_flat[0:1, bass.ds(pid * 256 + tt * 128, 128)])
                ps_zt = pssm.tile([128, 1], F32, tag="ps_sm", name="ps_zt")
                nc.tensor.transpose(ps_zt[:], ztv[:], ident_sb[:1, :1])
                ztc = wpool.tile([128, 1], F32, tag="ztc")
                nc.vector.tensor_copy(ztc[:], ps_zt[:])
                hzt = wpool.tile([128, H], F32, tag="hzt")
                nc.sync.dma_start(hzt[:], hz[tt * 128:(tt + 1) * 128, :])
                yz = wpool.tile([128, H], F32, tag="yz")
                nc.scalar.activation(yz[:], hzt[:], ACT_F.Copy,
                                     scale=ztc[:, 0:1])
                nc.sync.dma_start(partial2[tt * 128:(tt + 1) * 128, :], yz[:])

            # ========= phase 2: dispatch bookkeeping (tile-batched) =========
            mask4 = ppool.tile([128, TMAX, NJ, 1], F32, tag="mask4")
            gvt4 = ppool.tile([128, TMAX, NJ, 1], F32, tag="gvt4")
            eq4 = wpool.tile([128, TMAX, NJ, TOPK], F32, tag="eq4")
            nc.vector.tensor_tensor(
                eq4[:], idf_all[:].to_broadcast([128, TMAX, NJ, TOPK]),
                te_sb[:].to_broadcast([128, TMAX, NJ, TOPK]),
                op=AluOp.is_equal)
            gv4 = wpool.tile([128, TMAX, NJ, TOPK], F32, tag="gv4")
            nc.vector.tensor_mul(
                gv4[:], eq4[:], g_all[:].to_broadcast([128, TMAX, NJ, TOPK]))
            nc.vector.tensor_reduce(mask4[:], eq4[:], axis=AXL.X, op=AluOp.max)
            nc.vector.tensor_reduce(gvt4[:], gv4[:], axis=AXL.X, op=AluOp.add)
            mask_all = mask4[:].rearrange("p t j o -> p t (j o)")
            # gatings into the one-hot lhsT (transposed view write)
            for tau in range(TMAX):
                nc.vector.tensor_copy(
                    lhsT_all[:, :, 2 + tau],
                    gvt4[:, tau, :, :].rearrange("p j o -> p (j o)"))
            # ranks: rowsum per (tau), prefix over partitions, scan over j
            rowsum_all = wpool.tile([128, TMAX, 1], F32, tag="rowsumall")
            nc.vector.tensor_reduce(rowsum_all[:], mask4[:].rearrange(
                "p t j o -> p t (j o)"), axis=AXL.X, op=AluOp.add)
            ps_rp = pssm.tile([128, TMAX], F32, tag="ps_sm", name="ps_rp")
            nc.tensor.matmul(ps_rp[:], lhsT=uts_sb[:],
                             rhs=rowsum_all[:].rearrange("p t o -> p (t o)"),
                             start=True, stop=True)
            rp_sb = wpool.tile([128, TMAX, 1], F32, tag="rpsb")
            nc.vector.tensor_copy(rp_sb[:].rearrange("p t o -> p (t o)"),
                                  ps_rp[:])
            inrow_all = ppool.tile([128, TMAX, NJ], F32, tag="inrowall")
            for tau in range(TMAX):
                nc.vector.tensor_tensor_scan(
                    inrow_all[:, tau, :],
                    mask4[:, tau, :, :].rearrange("p j o -> p (j o)"),
                    zeros16[:], 0.0, op0=AluOp.add, op1=AluOp.add)
            pos = wpool.tile([128, TMAX, NJ], F32, tag="pos")
            nc.vector.tensor_tensor(
                pos[:], inrow_all[:],
                rp_sb[:].to_broadcast([128, TMAX, NJ]), op=AluOp.add)
            nc.vector.tensor_sub(pos[:], pos[:], mask_all)
            t1 = wpool.tile([128, TMAX, NJ], F32, tag="t1")
            nc.vector.tensor_tensor(
                t1[:], pos[:], tlo_sb[:].to_broadcast([128, TMAX, NJ]),
                op=AluOp.subtract)
            okr = wpool.tile([128, TMAX, NJ], F32, tag="okr")
            nc.vector.tensor_scalar(okr[:], t1[:], -0.5, None, op0=AluOp.is_gt)
            ok2 = wpool.tile([128, TMAX, NJ], F32, tag="ok2")
            nc.vector.tensor_scalar(ok2[:], t1[:], 127.5, None, op0=AluOp.is_lt)
            nc.vector.tensor_mul(okr[:], okr[:], ok2[:])
            nc.vector.tensor_mul(okr[:], okr[:], mask_all)
            oku = wpool.tile([128, TMAX, NJ], U8, tag="oku")
            nc.vector.tensor_copy(oku[:], okr[:])
            slotm = ppool.tile([128, TMAX, NJ], F32, tag="slotm")
            nc.vector.memset(slotm[:], -4.0)
            nc.vector.copy_predicated(slotm[:], oku[:], t1[:])

            # one-hot build + inversion matmul (PSUM: the big 2-bank half
            # shares the double-buffered pool with mm1's gu)
            invA = psbig.tile([2 + TMAX, 1024], F32, tag="ps_big",
                              name="ps_invA")
            invB = psib.tile([2 + TMAX, NB], F32, tag="ps_ib")
            for j in range(NJ):
                oh = wpool.tile([128, TMAX, 128], BF16, tag="oh")
                nc.vector.tensor_tensor(
                    oh[:], iota_sb[:].to_broadcast([128, TMAX, 128]),
                    slotm[:, :, j:j + 1].to_broadcast([128, TMAX, 128]),
                    op=AluOp.is_equal)
                ohf = oh[:].rearrange("p a b -> p (a b)")
                for lo, hi in ((0, 512), (512, 1024), (1024, NSLOT)):
                    dst = invA[:, lo:hi] if hi <= 1024 else invB[:, 0:NB]
                    nc.tensor.matmul(dst, lhsT=lhsT_all[:, j, :],
                                     rhs=ohf[:, lo:hi],
                                     start=(j == 0), stop=(j == NJ - 1))

            # decode via PE transpose: tsp[p, tau, :] = inv[:, tau*128+p]
            inv_sb = wpool.tile([2 + TMAX, NSLOT], F32, tag="invsb")
            nc.vector.tensor_copy(inv_sb[:, 0:1024], invA[:])
            nc.vector.tensor_copy(inv_sb[:, 1024:NSLOT], invB[:])
            tsp_sb = ppool.tile([128, TMAX, 2 + TMAX], F32, tag="tsp")
            for tau in range(TMAX):
                ps_tsp = pssm.tile([128, 128], F32, tag="ps_sm", name="ps_tsp")
                nc.tensor.transpose(ps_tsp[:, :2 + TMAX],
                                    inv_sb[:, tau * 128:(tau + 1) * 128],
                                    ident_sb[:2 + TMAX, :2 + TMAX])
                nc.vector.tensor_copy(tsp_sb[:, tau, :], ps_tsp[:, :2 + TMAX])
            # r = 128*hi + lo  (per-partition layout [p, tau])
            r_pt = ppool.tile([128, TMAX], F32, tag="rpt")
            nc.vector.scalar_tensor_tensor(r_pt[:], tsp_sb[:, :, 0], 128.0,
                                           tsp_sb[:, :, 1], op0=AluOp.mult,
                                           op1=AluOp.add)
            g_wr = ppool.tile([128, TMAX], F32, tag="gwr")
            for tau in range(TMAX):
                nc.vector.tensor_copy(g_wr[:, tau:tau + 1],
                                      tsp_sb[:, tau, 2 + tau:3 + tau])
            # slot->token list to DRAM already in the 16-wrapped DGE index
            # layout: slots_dram[p2, tau*8+q] = r[token partition q*16+p2]
            r_i16 = wpool.tile([128, TMAX], I16, tag="ri16")
            nc.vector.tensor_copy(r_i16[:], r_pt[:])
            slots_dram = dpool.tile([1, NSLOT], I16, tag="slotsdram")
            nc.sync.dma_start(
                slots_dram[0, :].rearrange("(t p) -> p t", p=128), r_i16[:])
            idxw = ppool.tile([128, NSLOT // 16], I16, tag="idxw")
            for grp in range(8):
                nc.sync.dma_start(
                    idxw[grp * 16:(grp + 1) * 16, :],
                    slots_dram[0, :].rearrange("(c p) -> p c", p=16))

            # =============== phase 3: gather + FFN + combine ===============
            # all transposing gathers first (gpsimd program order => no
            # per-tile gather stall)
            xt_tiles = []
            for tau in range(TMAX):
                xt = xtpool.tile([128, 8, 128], BF16, tag="xt")
                nc.gpsimd.dma_gather(
                    out_ap=xt[:], in_ap=hidden_rb[:],
                    idxs_ap=idxw[:, tau * 8:(tau + 1) * 8],
                    num_idxs=128, num_idxs_reg=128, elem_size=H,
                    transpose=True)
                xt_tiles.append(xt)

            for s in range(NSL):
                w13_sb = w13pool.tile([128, 8, 2 * I], BF16, tag="w13")
                w2_sb = w2pool.tile([128, 4, H], BF16, tag="w2")
                nc.sync.dma_start(w13_sb[:],
                                  w13s[s].rearrange("(k p) i -> p k i", p=128))
                nc.sync.dma_start(w2_sb[:],
                                  w2s[s].rearrange("(k p) i -> p k i", p=128))

                for tau in SLOT_TILES[s]:
                    xt = xt_tiles[tau]
                    # mm1
                    ps_gu = psbig.tile([128, 2 * I], F32, tag="ps_big",
                                       name="ps_gu")
                    for k in range(8):
                        for n in range(2):
                            nc.tensor.matmul(ps_gu[:, n * 512:(n + 1) * 512],
                                             lhsT=xt[:, k, :],
                                             rhs=w13_sb[:, k, n * 512:(n + 1) * 512],
                                             start=(k == 0), stop=(k == 7))
                    # h = silu(gate) * up = gate * sigmoid(gate) * up
                    sg = wpool.tile([128, I], F32, tag="sg")
                    nc.scalar.activation(sg[:], ps_gu[:, :I], ACT_F.Sigmoid)
                    nc.vector.tensor_mul(sg[:], sg[:], ps_gu[:, :I])
                    hh = wpool.tile([128, I], BF16, tag="hh")
                    nc.vector.tensor_mul(hh[:], sg[:], ps_gu[:, I:])
                    ht2 = wpool.tile([128, 4, 128], BF16, tag="ht2")
                    for k in range(4):
                        ps_t2 = pssm.tile([128, 128], BF16, tag="ps_sm",
                                          name="ps_t2")
                        nc.tensor.transpose(ps_t2[:],
                                            hh[:, k * 128:(k + 1) * 128],
                                            identb_sb[:])
                        nc.vector.tensor_copy(ht2[:, k, :], ps_t2[:])
                    # mm2
                    ps_y = psy.tile([128, H], F32, tag="ps_y")
                    for k in range(4):
                        for n in range(2):
                            nc.tensor.matmul(ps_y[:, n * 512:(n + 1) * 512],
                                             lhsT=ht2[:, k, :],
                                             rhs=w2_sb[:, k, n * 512:(n + 1) * 512],
                                             start=(k == 0), stop=(k == 3))
                    yv = wpool.tile([128, 1, H], BF16, tag="yv")
                    nc.scalar.activation(yv[:, 0, :], ps_y[:], ACT_F.Copy,
                                         scale=g_wr[:, tau:tau + 1])
                    nc.gpsimd.dma_scatter_add(
                        out_ap=partial[:], in_ap=yv[:],
                        idxs_ap=idxw[:, tau * 8:(tau + 1) * 8],
                        num_idxs=128, num_idxs_reg=128, elem_size=H)

    nc.compile()
    _NC_CACHE[key] = nc
    return nc


# ---------------------------------------------------------------------------
# host wrapper
# ---------------------------------------------------------------------------

def make_in_maps(hidden_states, router_w, e_score_correction_bias, w13, w2):
    import ml_dtypes
    hidden_states = np.asarray(hidden_states, np.float32)
    router_w = np.asarray(router_w, np.float32)
    bias = np.asarray(e_score_correction_bias, np.float32)
    w13 = np.asarray(w13, np.float32)
    w2 = np.asarray(w2, np.float32)

    ids = _host_routing(hidden_states, router_w, bias)
    caps, slot_expert, tiles = _schedule(ids)
    TMAX = sum(caps)
    NSL = len(caps)

    # r = (t % 128) * 16 + t // 128  <->  t = (r % 16) * 128 + r // 16
    r_of_t = (np.arange(T) % 128) * 16 + np.arange(T) // 128
    t_of_r = np.empty(T, np.int64)
    t_of_r[r_of_t] = np.arange(T)

    hidden_T = np.ascontiguousarray(hidden_states.T)
    hidden_rows = np.ascontiguousarray(hidden_states[t_of_r])
    hidden_rb = hidden_rows.astype(ml_dtypes.bfloat16)
    rwt = np.ascontiguousarray(router_w.T)
    bias_b = np.tile(bias[None, :], (128, 1))
    w13t = np.ascontiguousarray(
        w13.transpose(0, 2, 1)).astype(ml_dtypes.bfloat16)   # [e, h, 2I]
    w2t = np.ascontiguousarray(
        w2.transpose(0, 2, 1)).astype(ml_dtypes.bfloat16)    # [e, i, h]

    rr = np.arange(T).reshape(128, NJ).astype(np.float32)  # r at [p, j]
    rhl = np.stack([rr // 128, rr % 128], axis=-1).astype(ml_dtypes.bfloat16)
    iota128 = np.tile(np.arange(128, dtype=np.float32), (128, 1))
    ident = np.eye(128, dtype=np.float32)
    identb = np.eye(128, dtype=ml_dtypes.bfloat16)
    uts128 = np.triu(np.ones((128, 128), np.float32), k=1)

    in_maps = []
    for c in range(NCORES):
        te = np.array([tiles[c][tau][0] for tau in range(TMAX)], np.float32)
        tlo = np.array([tiles[c][tau][1] for tau in range(TMAX)], np.float32)
        in_maps.append({
            "hidden_T": hidden_T,
            "hidden_rb": hidden_rb,
            "rwt": rwt,
            "bias_b": bias_b,
            "w13s": np.ascontiguousarray(
                w13t[[slot_expert[c][s] for s in range(NSL)]]),
            "w2s": np.ascontiguousarray(
                w2t[[slot_expert[c][s] for s in range(NSL)]]),
            "tile_e": np.tile(te[None, :], (128, 1)),
            "tile_lo": np.tile(tlo[None, :], (128, 1)),
            "rhl": rhl,
            "iota128": iota128,
            "ident": ident,
            "identb": identb,
            "uts128": uts128,
            "hz": np.ascontiguousarray(hidden_rows[c * 256:(c + 1) * 256]),
        })
    return caps, in_maps, t_of_r


def kernel(hidden_states, router_w, e_score_correction_bias, w13, w2,
           _trace=False):
    caps, in_maps, t_of_r = make_in_maps(hidden_states, router_w,
                                         e_score_correction_bias, w13, w2)
    nc = build_nc(caps)
    res = run_bass_kernel_spmd(nc, in_maps, core_ids=list(range(NCORES)),
                               trace=_trace)
    total = np.zeros((T, H), np.float64)
    for c in range(NCORES):
        total += res.results[c]["partial"].astype(np.float64)
        total[c * 256:(c + 1) * 256] += \
            res.results[c]["partial2"].astype(np.float64)
    out = np.empty((T, H), np.float32)
    out[t_of_r] = total.astype(np.float32)      # out[t] = total[r(t)]
    kernel._last_results = res
    return out
